# revision 1
# baseline (speedup 1.0000x reference)
"""Trainium2 Bass kernel for nn_LlamaAttention_6588479832091.

Math notes:
  - The reference attention contracts q and k at the SAME sequence position
    (scores = einsum('bshd,bstd->bsht', q, k)), and RoPE applies the same
    orthogonal transform to q and k at equal positions, so RoPE cancels
    exactly: (P R q)·(P R k) = q·k.  v and the output path never see RoPE.
    The kernel therefore computes: q/k/v projections, per-token 16x16
    cross-head softmax attention, and the output projection.
  - Sharding: data-parallel over the 16384 tokens -> 2048 tokens per core,
    weights replicated.  No collectives.
  - Projections run as float32r matmuls (full PE speed, ~1.4e-4 rel err);
    the small attention matmuls run in plain fp32.

Layouts (host-prepared, all "partition-first" 3D):
  xt   [128, 16, 2048]  xt[p, ct, t] = x_shard[t, ct*128+p]           (f32r)
  wqt  [128, 16, 2048]  wqt[p, ct, m] = wq[m, ct*128+p] / sqrt(128)   (f32r)
  wkt, wvt: same layout as wqt (wk, wv unscaled)                      (f32r)
  wot  [128, 16, 2048]  wot[p, mt, r] = wo[r, mt*128+p]               (f32r)
  mask [128, 128]       0 where p%8 == n%8 else -30000                (f32)
  ident[128, 128]       identity                                      (f32)
  ot   [128, 16, 2048]  ot[p, rt, t] = out_shard[t, rt*128+p]         (f32, output)
"""
import sys

for _p in ("/opt/trn_rl_repo", "/root/.axon_site/_ro/trn_rl_repo"):
    if _p not in sys.path:
        sys.path.insert(0, _p)

import numpy as np

T_CORE = 2048      # tokens per core
N_CORES = 8
H = 16             # heads
HD = 128           # head dim
HIDDEN = 2048
CT = HIDDEN // 128  # 16 contraction tiles
TCH = 512          # token chunk for N=512 matmuls
SUB = 128          # attention sub-chunk tokens
ATW = 256          # attn-output tile width (written by 2 subs)
GRP = 8            # tokens per attention group

_CACHED = {}


def _build(phases="PAO"):
    import concourse.mybir as mybir
    import concourse.tile as tile
    import concourse.bacc as bacc

    f32 = mybir.dt.float32
    f32r = mybir.dt.float32r

    nc = bacc.Bacc("TRN2", target_bir_lowering=False, debug=False)

    xt = nc.declare_dram_parameter("xt", [128, CT, T_CORE], f32r, isOutput=False)
    wqt = nc.declare_dram_parameter("wqt", [128, CT, HIDDEN], f32r, isOutput=False)
    wkt = nc.declare_dram_parameter("wkt", [128, CT, HIDDEN], f32r, isOutput=False)
    wvt = nc.declare_dram_parameter("wvt", [128, CT, HIDDEN], f32r, isOutput=False)
    wot = nc.declare_dram_parameter("wot", [128, CT, HIDDEN], f32r, isOutput=False)
    maskd = nc.declare_dram_parameter("maskd", [128, 512], f32, isOutput=False)
    identd = nc.declare_dram_parameter("identd", [128, 128], f32, isOutput=False)
    ot = nc.declare_dram_parameter("ot", [128, CT, T_CORE], f32, isOutput=True)

    with tile.TileContext(nc) as tc:
        with tc.tile_pool(name="dram", bufs=1, space="DRAM") as dram:
            NTCH = T_CORE // TCH
            qT = [dram.tile([128, H, TCH], f32, name=f"qT{i}") for i in range(NTCH)]
            kT = [dram.tile([128, H, TCH], f32, name=f"kT{i}") for i in range(NTCH)]
            vT = [dram.tile([128, H, TCH], f32, name=f"vT{i}") for i in range(NTCH)]

            # ---------------- Phase P: q/k/v projections (f32r) -------------
            if "P" in phases:
             with tc.tile_pool(name="p_x", bufs=1) as p_x, \
                 tc.tile_pool(name="p_w", bufs=4) as p_w, \
                 tc.tile_pool(name="p_ev", bufs=6) as p_ev, \
                 tc.tile_pool(name="p_ps", bufs=6, space="PSUM") as p_ps:
                x_sb = p_x.tile([128, CT, T_CORE], f32r)
                for ct in range(CT):
                    nc.gpsimd.dma_start(x_sb[:, ct, :], xt[:, ct, :])
                for wsrc, qdst in ((wqt, qT), (wkt, kT), (wvt, vT)):
                    for mt in range(H):
                        wslab = p_w.tile([128, CT, 128], f32r, tag="wslab")
                        nc.sync.dma_start(wslab[:], wsrc[:, :, mt * 128:(mt + 1) * 128])
                        for tch in range(T_CORE // TCH):
                            psum = p_ps.tile([128, TCH], f32, tag="pp")
                            for kt in range(CT):
                                nc.tensor.matmul(
                                    psum[:],
                                    wslab[:, kt, :],
                                    x_sb[:, kt, tch * TCH:(tch + 1) * TCH],
                                    start=(kt == 0), stop=(kt == CT - 1))
                            ev = p_ev.tile([128, TCH], f32, tag="ev")
                            nc.vector.tensor_copy(ev[:], psum[:])
                            nc.sync.dma_start(qdst[tch][:, mt, :], ev[:])

            # ---------------- Phase A: cross-head attention ------------------
            # 4 groups (32 tokens) per "macro": wide DVE/ACT ops, software-
            # skewed emission so the in-order PE stream never stalls on the
            # current macro's exp.
            if "A" in phases:
             with tc.tile_pool(name="a_io", bufs=1) as a_io, \
                 tc.tile_pool(name="a_wk", bufs=3) as a_wk, \
                 tc.tile_pool(name="a_ps", bufs=1, space="PSUM") as a_ps:
                mask_sb = a_io.tile([128, 512], f32, tag="mask")
                ident_sb = a_io.tile([128, 128], f32, tag="ident")
                ones_sb = a_io.tile([128, 1], f32, tag="ones")
                nc.sync.dma_start(mask_sb[:], maskd[:])
                nc.sync.dma_start(ident_sb[:], identd[:])
                nc.gpsimd.memset(ones_sb[:], 1.0)
                NG = SUB // GRP          # groups per sub-chunk
                NM = NG // 4             # macros per sub-chunk
                MAC = 4 * GRP            # tokens per macro

                def stage1(st, m):
                    """MM1 x4 + mask + exp for macro m."""
                    ps_s = a_ps.tile([128, 512], f32, tag="s", bufs=2)
                    for i in range(4):
                        g = 4 * m + i
                        nc.tensor.matmul(ps_s[:, i * 128:(i + 1) * 128],
                                         st["k"][:, g, :], st["q"][:, g, :],
                                         start=True, stop=True)
                    nc.vector.tensor_add(ps_s[:], ps_s[:], mask_sb[:])
                    wt = a_wk.tile([128, 512], f32, tag="wt_sb", bufs=4)
                    nc.scalar.activation(wt[:], ps_s[:],
                                         mybir.ActivationFunctionType.Exp)
                    st[("wt", m)] = wt

                def stage2(st, m):
                    """Z + rz + V-transpose + vp evac for macro m."""
                    wt = st[("wt", m)]
                    ps_z = a_ps.tile([128, 4], f32, tag="z", bufs=1)
                    for i in range(4):
                        nc.tensor.matmul(ps_z[:, i:i + 1],
                                         wt[:, i * 128:(i + 1) * 128], ones_sb[:],
                                         start=True, stop=True)
                    rz = a_wk.tile([128, 4], f32, tag="rz", bufs=3)
                    nc.vector.reciprocal(rz[:], ps_z[:])
                    st[("rz", m)] = rz
                    ps_v = a_ps.tile([128, 512], f32, tag="vp", bufs=1)
                    for i in range(4):
                        g = 4 * m + i
                        nc.tensor.transpose(ps_v[:, i * 128:(i + 1) * 128],
                                            st["v"][:, g, :], ident_sb[:])
                    vp = a_wk.tile([128, 512], f32, tag="vp_sb", bufs=4)
                    nc.vector.tensor_copy(vp[:], ps_v[:])
                    st[("vp", m)] = vp

                def stage3(st, m):
                    """MM2 + normalize for macro m."""
                    wt = st.pop(("wt", m))
                    vp = st.pop(("vp", m))
                    rz = st.pop(("rz", m))
                    ps_at = a_ps.tile([128, 512], f32, tag="attn", bufs=3)
                    for i in range(4):
                        nc.tensor.matmul(ps_at[:, i * 128:(i + 1) * 128],
                                         wt[:, i * 128:(i + 1) * 128],
                                         vp[:, i * 128:(i + 1) * 128],
                                         start=True, stop=True)
                    an = a_wk.tile([128, 512], f32, tag="an_sb", bufs=4)
                    nc.vector.tensor_mul(
                        an[:].rearrange("p (g c) -> p g c", g=4),
                        ps_at[:].rearrange("p (g c) -> p g c", g=4),
                        rz[:].broadcast_to((128, 4, 128)))
                    st[("an", m)] = an

                def stage4(st, m):
                    """aT transpose + at-copy for macro m."""
                    an = st.pop(("an", m))
                    ps_aT = a_ps.tile([128, 512], f32, tag="aTp", bufs=1)
                    for i in range(4):
                        nc.tensor.transpose(ps_aT[:, i * 128:(i + 1) * 128],
                                            an[:, i * 128:(i + 1) * 128], ident_sb[:])
                    nc.scalar.copy(
                        st["at"][:, :, m * MAC:(m + 1) * MAC].rearrange(
                            "p h (g ti) -> p g h ti", g=4),
                        ps_aT[:].rearrange("p (g h ti) -> p g h ti", g=4, h=H))

                prev_at = [None, None]
                for sub in range(T_CORE // SUB):
                    sl = slice(sub * SUB, (sub + 1) * SUB)
                    st = {}
                    st["q"] = a_io.tile([128, NG, 128], f32, tag="q", bufs=2, name="qpk")
                    st["k"] = a_io.tile([128, NG, 128], f32, tag="k", bufs=2, name="kpk")
                    st["v"] = a_io.tile([128, NG, 128], f32, tag="v", bufs=2, name="vpk")
                    if sub % 2 == 0:
                        at_full = a_io.tile([128, H, ATW], f32r, tag="at", bufs=3,
                                            name="atsb")
                        prev_at = [prev_at[1], at_full]
                    st["at"] = prev_at[1][:, :, (sub % 2) * SUB:(sub % 2) * SUB + SUB]
                    tch_i = sub * SUB // TCH
                    half = (sub * SUB) % TCH
                    for nm, pk, srcd, eng in (("q", st["q"], qT, nc.vector),
                                              ("k", st["k"], kT, nc.scalar),
                                              ("v", st["v"], vT, nc.scalar)):
                        stg = a_io.tile([128, H, SUB], f32, tag=f"stg_{nm}",
                                        bufs=2, name=f"stg{nm}")
                        nc.sync.dma_start(stg[:], srcd[tch_i][:, :, half:half + SUB])
                        dst = pk[:].rearrange("p g (h tj) -> p g h tj", tj=GRP)
                        srcv = stg[:].rearrange("p h (g tj) -> p g h tj", tj=GRP)
                        if eng is nc.vector:
                            nc.vector.tensor_copy(dst, srcv)
                        else:
                            nc.scalar.copy(dst, srcv)
                    for m in range(NM + 3):
                        if m < NM:
                            stage1(st, m)
                        if 1 <= m <= NM:
                            stage2(st, m - 1)
                        if 2 <= m <= NM + 1:
                            stage3(st, m - 2)
                        if 3 <= m <= NM + 2:
                            stage4(st, m - 3)
                    if sub % 4 == 3:
                        tch = sub // 4
                        at_pair = (prev_at[0], prev_at[1])
                        for rt in range(CT):
                            woslab = a_io.tile([128, CT, 128], f32r,
                                               tag="woslab", bufs=3, name="woslab")
                            nc.sync.dma_start(
                                woslab[:], wot[:, :, rt * 128:(rt + 1) * 128])
                            po = a_ps.tile([128, TCH], f32, tag="attn", bufs=3)
                            for kt in range(CT):
                                nc.tensor.matmul(
                                    po[:, :ATW],
                                    woslab[:, kt, :],
                                    at_pair[0][:, kt, :],
                                    start=(kt == 0), stop=False)
                                nc.tensor.matmul(
                                    po[:, ATW:],
                                    woslab[:, kt, :],
                                    at_pair[1][:, kt, :],
                                    start=False, stop=(kt == CT - 1))
                            oev = a_io.tile([128, TCH], f32, tag="oev",
                                            bufs=4, name="oev")
                            nc.vector.tensor_copy(oev[:], po[:])
                            nc.sync.dma_start(
                                ot[:, rt, tch * TCH:(tch + 1) * TCH], oev[:])
    nc.compile()
    return nc


def _host_prep(x, wq, wk, wv, wo):
    """Build per-core input maps (layout transforms only)."""
    x2 = np.ascontiguousarray(x.reshape(-1, HIDDEN))          # (16384, 2048)
    wqs = (wq / np.sqrt(np.float32(HD))).astype(np.float32)

    def wt3(w):   # (m, c) weight -> [128, CT, HIDDEN] with w.T tiled on c
        wt = np.ascontiguousarray(w.T)                        # (c, m)
        return np.ascontiguousarray(
            wt.reshape(CT, 128, HIDDEN).transpose(1, 0, 2))

    wqt, wkt, wvt, wot = wt3(wqs), wt3(wk), wt3(wv), wt3(wo)
    p = np.arange(128)[:, None]
    n = np.arange(128)[None, :]
    mask = np.where((p % GRP) == (n % GRP), 0.0, -30000.0).astype(np.float32)
    mask = np.tile(mask, (1, 4))
    ident = np.eye(128, dtype=np.float32)

    in_maps = []
    for c in range(N_CORES):
        xs = x2[c * T_CORE:(c + 1) * T_CORE]                  # (2048, 2048)
        xt = np.ascontiguousarray(
            xs.T.reshape(CT, 128, T_CORE).transpose(1, 0, 2))
        in_maps.append({"xt": xt, "wqt": wqt, "wkt": wkt, "wvt": wvt,
                        "wot": wot, "maskd": mask, "identd": ident})
    return in_maps


def kernel(x, wq, wk, wv, wo, inv_freq):
    # inv_freq is unused: RoPE is an identical orthogonal transform on q and k
    # at equal positions, and this attention only contracts same-position q·k,
    # so it cancels exactly (verified vs the fp32 reference: ~6e-6 rel).
    from concourse.bass_utils import run_bass_kernel_spmd

    x = np.asarray(x, dtype=np.float32)
    wq = np.asarray(wq, dtype=np.float32)
    wk = np.asarray(wk, dtype=np.float32)
    wv = np.asarray(wv, dtype=np.float32)
    wo = np.asarray(wo, dtype=np.float32)

    if "nc" not in _CACHED:
        _CACHED["nc"] = _build()
    nc = _CACHED["nc"]

    in_maps = _host_prep(x, wq, wk, wv, wo)
    res = run_bass_kernel_spmd(nc, in_maps, core_ids=list(range(N_CORES)))

    out = np.empty((N_CORES * T_CORE, HIDDEN), dtype=np.float32)
    for c in range(N_CORES):
        ot = res.results[c]["ot"]                              # (128, 16, 2048)
        out[c * T_CORE:(c + 1) * T_CORE] = (
            ot.transpose(2, 1, 0).reshape(T_CORE, HIDDEN))
    return out.reshape(x.shape[0], x.shape[1], HIDDEN)



# revision 12
# speedup vs baseline: 1.1113x; 1.1113x over previous
"""Trainium2 Bass kernel for nn_LlamaAttention_6588479832091.

Math notes:
  - The reference attention contracts q and k at the SAME sequence position
    (scores = einsum('bshd,bstd->bsht', q, k)), and RoPE applies the same
    orthogonal transform to q and k at equal positions, so RoPE cancels
    exactly: (P R q)·(P R k) = q·k.  v and the output path never see RoPE.
    The kernel therefore computes: q/k/v projections, per-token 16x16
    cross-head softmax attention, and the output projection.
  - Sharding: data-parallel over the 16384 tokens -> 2048 tokens per core,
    weights replicated.  No collectives.
  - All matmuls run in bf16 (1 cycle/row on the PE; fp32 would be 4) with
    fp32 PSUM accumulation.  End-to-end rel err ~5e-3, tolerance is 2e-2.
  - Fully fused per-512-token-chunk pipeline: the q/k/v projection psums are
    evacuated DIRECTLY into the attention's group-packed SBUF layout (no
    DRAM roundtrip, no staging loads).  Weight slabs are re-streamed per
    chunk instead (DMA is far below the PE roofline).  Emission order
    proj(0), proj(1), A(0), proj(2), A(1), proj(3), A(2), A(3) keeps the
    PE busy across chunk boundaries.
  - Attention softmax work is spread over DVE/ACT/Pool so no single engine
    exceeds the PE's per-macro cadence: exp on ACT, mask-mul + recip +
    normalize on DVE, v-transpose evac on Pool, attn-transpose evac split
    ACT/Pool.  Mask is multiplicative (0/1) applied to exp(scores); scores
    are O(few) so exp never overflows.

Layouts (host-prepared, all partition-first, bf16):
  xt   [128, 4, 8192]   xt[p, t, kt*512+i] = x_shard[t*512+i, kt*128+p]
  wq4  [128, 16, 2048]  wq4[p, mt, kt*128+j] = wq[mt*128+j, kt*128+p]/sqrt(128)
  wk4, wv4: same layout as wq4 (wk, wv unscaled)
  wo4  [128, 16, 2048]  wo4[p, rt, kt*128+j] = wo[rt*128+j, kt*128+p]
  maskd [128, 512]      1 where p%8 == n%8 else 0 (tiled x4 groups)
  identd [128, 128]     identity
  otb  [128, 16, 2048]  otb[p, rt, t] = out_shard[t, rt*128+p]   (output)
"""
import sys

for _p in ("/opt/trn_rl_repo", "/root/.axon_site/_ro/trn_rl_repo"):
    if _p not in sys.path:
        sys.path.insert(0, _p)

import numpy as np

T_CORE = 2048      # tokens per core
N_CORES = 8
H = 16             # heads
HD = 128           # head dim
HIDDEN = 2048
CT = HIDDEN // 128  # 16 contraction tiles
TCH = 512          # tokens per fused chunk
NTCH = T_CORE // TCH  # 4 chunks
GRP = 8            # tokens per attention group
NG = TCH // GRP    # 64 groups per chunk
MAC = 32           # tokens per macro (4 groups)
NMAC = TCH // MAC  # 16 macros per chunk

_CACHED = {}


def _build():
    import concourse.mybir as mybir
    import concourse.tile as tile
    import concourse.bacc as bacc

    f32 = mybir.dt.float32
    bf16 = mybir.dt.bfloat16
    EXP = mybir.ActivationFunctionType.Exp

    nc = bacc.Bacc("TRN2", target_bir_lowering=False, debug=False)

    xt = nc.declare_dram_parameter("xt", [128, NTCH, CT * TCH], bf16, isOutput=False)
    wq4 = nc.declare_dram_parameter("wq4", [128, H, CT * 128], bf16, isOutput=False)
    wk4 = nc.declare_dram_parameter("wk4", [128, H, CT * 128], bf16, isOutput=False)
    wv4 = nc.declare_dram_parameter("wv4", [128, H, CT * 128], bf16, isOutput=False)
    wo4 = nc.declare_dram_parameter("wo4", [128, CT, CT * 128], bf16, isOutput=False)
    maskd = nc.declare_dram_parameter("maskd", [128, 512], bf16, isOutput=False)
    identd = nc.declare_dram_parameter("identd", [128, 128], bf16, isOutput=False)
    otb = nc.declare_dram_parameter("otb", [128, CT, T_CORE], bf16, isOutput=True)

    with tile.TileContext(nc) as tc:
        with tc.tile_pool(name="io", bufs=1) as io, \
             tc.tile_pool(name="wp", bufs=1) as wp, \
             tc.tile_pool(name="xp", bufs=1) as xp, \
             tc.tile_pool(name="qk", bufs=1) as qkp, \
             tc.tile_pool(name="aw", bufs=1) as aw, \
             tc.tile_pool(name="ps", bufs=1, space="PSUM") as psp:

            mask_sb = io.tile([128, 512], bf16, name="masksb")
            ident_sb = io.tile([128, 128], bf16, name="identsb")
            ones_sb = io.tile([128, 1], bf16, name="onessb")
            nc.sync.dma_start(mask_sb[:], maskd[:])
            nc.sync.dma_start(ident_sb[:], identd[:])
            nc.gpsimd.memset(ones_sb[:], 1.0)

            def proj_chunk(t):
                """q/k/v projections for 512 tokens, evacuated straight into
                the attention's packed layout [128=d, group, (h, tj)]."""
                x_sb = xp.tile([128, CT * TCH], bf16, tag="x", bufs=2, name="xsb")
                # split x load so the first psum's kt-chain can start early
                for piece in range(4):
                    sl = slice(piece * 4 * TCH, (piece + 1) * 4 * TCH)
                    nc.sync.dma_start(x_sb[:, sl], xt[:, t, sl])
                pk = {}
                for wname, wsrc in (("q", wq4), ("k", wk4), ("v", wv4)):
                    dst = qkp.tile([128, NG, 128], bf16, tag=f"{wname}pk",
                                   bufs=2, name=f"{wname}pk")
                    pk[wname] = dst
                    for mt in range(H):
                        wslab = wp.tile([128, CT * 128], bf16, tag="wslab",
                                        bufs=6, name="wslab")
                        nc.sync.dma_start(wslab[:], wsrc[:, mt, :])
                        pp = psp.tile([128, TCH], f32, tag="big", bufs=2,
                                      name="pp")
                        for kt in range(CT):
                            nc.tensor.matmul(
                                pp[:],
                                wslab[:, kt * 128:(kt + 1) * 128],
                                x_sb[:, kt * TCH:(kt + 1) * TCH],
                                start=(kt == 0), stop=(kt == CT - 1))
                        nc.vector.tensor_copy(
                            dst[:, :, mt * GRP:(mt + 1) * GRP],
                            pp[:].rearrange("p (g tj) -> p g tj", tj=GRP))
                return pk

            def attn_chunk(t, pk):
                """Cross-head attention + output projection for one chunk."""
                qpk, kpk, vpk = pk["q"], pk["k"], pk["v"]
                at = aw.tile([128, CT, TCH], bf16, tag="at", bufs=1, name="atsb")
                st = {}

                def stage1(m):
                    ps_s = psp.tile([128, 512], f32, tag="s", bufs=2, name="ps_s")
                    for i in range(4):
                        g = 4 * m + i
                        nc.tensor.matmul(ps_s[:, i * 128:(i + 1) * 128],
                                         kpk[:, g, :], qpk[:, g, :],
                                         start=True, stop=True)
                    wt0 = aw.tile([128, 512], bf16, tag="wt0", bufs=3, name="wt0")
                    nc.scalar.activation(wt0[:], ps_s[:], EXP)
                    st[("wt0", m)] = wt0

                def stage1b(m):
                    # mask on Pool (SBUF-only engine) to offload DVE/ACT
                    wt0 = st.pop(("wt0", m))
                    wt = aw.tile([128, 512], bf16, tag="wt", bufs=3, name="wt")
                    nc.gpsimd.tensor_mul(wt[:], wt0[:], mask_sb[:])
                    st[("wt", m)] = wt

                def stage2(m):
                    wt = st[("wt", m)]
                    zt = psp.tile([128, TCH], f32, tag="big", bufs=2, name="zt")
                    for i in range(4):
                        nc.tensor.matmul(zt[:, i:i + 1],
                                         wt[:, i * 128:(i + 1) * 128], ones_sb[:],
                                         start=True, stop=True)
                    rz = aw.tile([128, 4], f32, tag="rz", bufs=3, name="rz")
                    nc.vector.reciprocal(rz[:], zt[:, :4])
                    st[("rz", m)] = rz
                    ps_v = psp.tile([128, 512], bf16, tag="v", bufs=1, name="ps_v")
                    for i in range(4):
                        g = 4 * m + i
                        nc.tensor.transpose(ps_v[:, i * 128:(i + 1) * 128],
                                            vpk[:, g, :], ident_sb[:])
                    vp = aw.tile([128, 512], bf16, tag="vp", bufs=3, name="vp")
                    nc.vector.tensor_copy(vp[:], ps_v[:])
                    st[("vp", m)] = vp

                def stage3(m):
                    wt = st.pop(("wt", m))
                    vp = st.pop(("vp", m))
                    rz = st.pop(("rz", m))
                    ps_at = psp.tile([128, 512], f32, tag="pat", bufs=2,
                                     name="ps_at")
                    for i in range(4):
                        nc.tensor.matmul(ps_at[:, i * 128:(i + 1) * 128],
                                         wt[:, i * 128:(i + 1) * 128],
                                         vp[:, i * 128:(i + 1) * 128],
                                         start=True, stop=True)
                    an = aw.tile([128, 512], bf16, tag="an", bufs=3, name="an")
                    nc.vector.tensor_mul(
                        an[:].rearrange("p (g c) -> p g c", g=4),
                        ps_at[:].rearrange("p (g c) -> p g c", g=4),
                        rz[:].broadcast_to((128, 4, 128)))
                    st[("an", m)] = an

                def stage4(m):
                    an = st.pop(("an", m))
                    ps_aT = psp.tile([128, 512], bf16, tag="aT", bufs=1,
                                     name="ps_aT")
                    for i in range(4):
                        nc.tensor.transpose(ps_aT[:, i * 128:(i + 1) * 128],
                                            an[:, i * 128:(i + 1) * 128],
                                            ident_sb[:])
                    # evac to at[d, h, tok] on ACT
                    nc.scalar.copy(
                        at[:, :, m * MAC:(m + 1) * MAC].rearrange(
                            "p h (g ti) -> p g h ti", ti=GRP),
                        ps_aT[:].rearrange(
                            "p (g h ti) -> p g h ti", g=4, h=H))

                for m in range(NMAC + 4):
                    if m < NMAC:
                        stage1(m)
                    if 1 <= m <= NMAC:
                        stage1b(m - 1)
                    if 2 <= m <= NMAC + 1:
                        stage2(m - 2)
                    if 3 <= m <= NMAC + 2:
                        stage3(m - 3)
                    if 4 <= m <= NMAC + 3:
                        stage4(m - 4)

                # output projection for this chunk
                for rt in range(CT):
                    woslab = wp.tile([128, CT * 128], bf16, tag="woslab",
                                     bufs=3, name="woslab")
                    nc.sync.dma_start(woslab[:], wo4[:, rt, :])
                    po = psp.tile([128, TCH], f32, tag="big", bufs=2, name="po")
                    for kt in range(CT):
                        nc.tensor.matmul(
                            po[:],
                            woslab[:, kt * 128:(kt + 1) * 128],
                            at[:, kt, :],
                            start=(kt == 0), stop=(kt == CT - 1))
                    oev = aw.tile([128, TCH], bf16, tag="oev", bufs=3, name="oev")
                    nc.vector.tensor_copy(oev[:], po[:])
                    nc.gpsimd.dma_start(
                        otb[:, rt, t * TCH:(t + 1) * TCH], oev[:])

            pks = {0: proj_chunk(0), 1: proj_chunk(1)}
            attn_chunk(0, pks.pop(0))
            pks[2] = proj_chunk(2)
            attn_chunk(1, pks.pop(1))
            pks[3] = proj_chunk(3)
            attn_chunk(2, pks.pop(2))
            attn_chunk(3, pks.pop(3))

    nc.compile()
    return nc


def _host_prep(x, wq, wk, wv, wo):
    """Build per-core input maps (layout transforms + bf16 casts only)."""
    import ml_dtypes
    bf16 = ml_dtypes.bfloat16

    x2 = np.ascontiguousarray(x.reshape(-1, HIDDEN))          # (16384, 2048)
    wqs = (wq / np.sqrt(np.float32(HD))).astype(np.float32)

    def wt4(w):   # [128, 16, 2048]: wt4[p, mt, kt*128+j] = w[mt*128+j, kt*128+p]
        return np.ascontiguousarray(
            w.reshape(H, 128, CT, 128).transpose(3, 0, 2, 1)
        ).reshape(128, H, CT * 128).astype(bf16)

    wq4, wk4, wv4, wo4 = wt4(wqs), wt4(wk), wt4(wv), wt4(wo)
    p = np.arange(128)[:, None]
    n = np.arange(128)[None, :]
    mask = np.where((p % GRP) == (n % GRP), 1.0, 0.0).astype(bf16)
    mask = np.tile(mask, (1, 4))
    ident = np.eye(128, dtype=np.float32).astype(bf16)

    in_maps = []
    for c in range(N_CORES):
        xs = x2[c * T_CORE:(c + 1) * T_CORE]                  # (2048, 2048)
        xtc = np.ascontiguousarray(
            xs.reshape(NTCH, TCH, CT, 128).transpose(3, 0, 2, 1)
        ).reshape(128, NTCH, CT * TCH).astype(bf16)
        in_maps.append({"xt": xtc, "wq4": wq4, "wk4": wk4, "wv4": wv4,
                        "wo4": wo4, "maskd": mask, "identd": ident})
    return in_maps


def kernel(x, wq, wk, wv, wo, inv_freq):
    # inv_freq is unused: RoPE is an identical orthogonal transform on q and k
    # at equal positions, and this attention only contracts same-position q·k,
    # so it cancels exactly.
    from concourse.bass_utils import run_bass_kernel_spmd

    x = np.asarray(x, dtype=np.float32)
    wq = np.asarray(wq, dtype=np.float32)
    wk = np.asarray(wk, dtype=np.float32)
    wv = np.asarray(wv, dtype=np.float32)
    wo = np.asarray(wo, dtype=np.float32)

    if "nc" not in _CACHED:
        _CACHED["nc"] = _build()
    nc = _CACHED["nc"]

    in_maps = _host_prep(x, wq, wk, wv, wo)
    res = run_bass_kernel_spmd(nc, in_maps, core_ids=list(range(N_CORES)))

    out = np.empty((N_CORES * T_CORE, HIDDEN), dtype=np.float32)
    for c in range(N_CORES):
        ot = np.asarray(res.results[c]["otb"]).astype(np.float32)  # (128,16,2048)
        out[c * T_CORE:(c + 1) * T_CORE] = (
            ot.transpose(2, 1, 0).reshape(T_CORE, HIDDEN))
    return out.reshape(x.shape[0], x.shape[1], HIDDEN)


# revision 15
# speedup vs baseline: 1.1214x; 1.0091x over previous
"""Trainium2 Bass kernel for nn_LlamaAttention_6588479832091.

Math notes:
  - The reference attention contracts q and k at the SAME sequence position
    (scores = einsum('bshd,bstd->bsht', q, k)), and RoPE applies the same
    orthogonal transform to q and k at equal positions, so RoPE cancels
    exactly: (P R q)·(P R k) = q·k.  v and the output path never see RoPE.
    The kernel therefore computes: q/k/v projections, per-token 16x16
    cross-head softmax attention, and the output projection.
  - Sharding: data-parallel over the 16384 tokens -> 2048 tokens per core,
    weights replicated.  No collectives.
  - All matmuls run in bf16 (1 cycle/row on the PE; fp32 would be 4) with
    fp32 PSUM accumulation.  End-to-end rel err ~5e-3, tolerance is 2e-2.
  - Fully fused per-512-token-chunk pipeline: the q/k/v projection psums are
    evacuated DIRECTLY into the attention's group-packed SBUF layout (no
    DRAM roundtrip, no staging loads).  Weight slabs are re-streamed per
    chunk instead (DMA is far below the PE roofline).  Emission order
    proj(0), proj(1), A(0), proj(2), A(1), proj(3), A(2), A(3) keeps the
    PE busy across chunk boundaries.
  - Attention softmax work is spread over DVE/ACT/Pool so no single engine
    exceeds the PE's per-macro cadence: exp on ACT, mask-mul + recip +
    normalize on DVE, v-transpose evac on Pool, attn-transpose evac split
    ACT/Pool.  Mask is multiplicative (0/1) applied to exp(scores); scores
    are O(few) so exp never overflows.

Layouts (host-prepared, all partition-first, bf16):
  xt   [128, 4, 8192]   xt[p, t, kt*512+i] = x_shard[t*512+i, kt*128+p]
  wq4  [128, 16, 2048]  wq4[p, mt, kt*128+j] = wq[mt*128+j, kt*128+p]/sqrt(128)
  wk4, wv4: same layout as wq4 (wk, wv unscaled)
  wo4  [128, 16, 2048]  wo4[p, rt, kt*128+j] = wo[rt*128+j, kt*128+p]
  maskd [128, 512]      1 where p%8 == n%8 else 0 (tiled x4 groups)
  identd [128, 128]     identity
  otb  [128, 16, 2048]  otb[p, rt, t] = out_shard[t, rt*128+p]   (output)
"""
import sys

for _p in ("/opt/trn_rl_repo", "/root/.axon_site/_ro/trn_rl_repo"):
    if _p not in sys.path:
        sys.path.insert(0, _p)

import numpy as np

T_CORE = 2048      # tokens per core
N_CORES = 8
H = 16             # heads
HD = 128           # head dim
HIDDEN = 2048
CT = HIDDEN // 128  # 16 contraction tiles
TCH = 512          # tokens per fused chunk
NTCH = T_CORE // TCH  # 4 chunks
GRP = 8            # tokens per attention group
NG = TCH // GRP    # 64 groups per chunk
MAC = 32           # tokens per macro (4 groups)
NMAC = TCH // MAC  # 16 macros per chunk

_CACHED = {}


def _build():
    import concourse.mybir as mybir
    import concourse.tile as tile
    import concourse.bacc as bacc

    f32 = mybir.dt.float32
    bf16 = mybir.dt.bfloat16
    EXP = mybir.ActivationFunctionType.Exp

    nc = bacc.Bacc("TRN2", target_bir_lowering=False, debug=False)

    xt = nc.declare_dram_parameter("xt", [128, NTCH, CT * TCH], bf16, isOutput=False)
    wq4 = nc.declare_dram_parameter("wq4", [128, H, CT * 128], bf16, isOutput=False)
    wk4 = nc.declare_dram_parameter("wk4", [128, H, CT * 128], bf16, isOutput=False)
    wv4 = nc.declare_dram_parameter("wv4", [128, H, CT * 128], bf16, isOutput=False)
    wo4 = nc.declare_dram_parameter("wo4", [128, CT, CT * 128], bf16, isOutput=False)
    maskd = nc.declare_dram_parameter("maskd", [128, 512], bf16, isOutput=False)
    identd = nc.declare_dram_parameter("identd", [128, 128], bf16, isOutput=False)
    otb = nc.declare_dram_parameter("otb", [128, CT, T_CORE], bf16, isOutput=True)

    with tile.TileContext(nc) as tc:
        with tc.tile_pool(name="io", bufs=1) as io, \
             tc.tile_pool(name="wp", bufs=1) as wp, \
             tc.tile_pool(name="xp", bufs=1) as xp, \
             tc.tile_pool(name="qk", bufs=1) as qkp, \
             tc.tile_pool(name="aw", bufs=1) as aw, \
             tc.tile_pool(name="ps", bufs=1, space="PSUM") as psp:

            mask_sb = io.tile([128, 512], bf16, name="masksb")
            ident_sb = io.tile([128, 128], bf16, name="identsb")
            ones_sb = io.tile([128, 1], bf16, name="onessb")
            nc.scalar.dma_start(mask_sb[:], maskd[:])
            nc.scalar.dma_start(ident_sb[:], identd[:])
            nc.gpsimd.memset(ones_sb[:], 1.0)

            def proj_chunk(t):
                """q/k/v projections for 512 tokens, evacuated straight into
                the attention's packed layout [128=d, group, (h, tj)]."""
                x_sb = xp.tile([128, CT * TCH], bf16, tag="x", bufs=2, name="xsb")
                # x on the ACT DMA queue (parallel with slab loads on sync);
                # split so the first psum's kt-chain can start early
                for piece in range(4):
                    sl = slice(piece * 4 * TCH, (piece + 1) * 4 * TCH)
                    nc.scalar.dma_start(x_sb[:, sl], xt[:, t, sl])
                pk = {}
                for wname, wsrc in (("q", wq4), ("k", wk4), ("v", wv4)):
                    dst = qkp.tile([128, NG, 128], bf16, tag=f"{wname}pk",
                                   bufs=2, name=f"{wname}pk")
                    pk[wname] = dst
                    for mt2 in range(H // 2):
                        # two head-slabs per DMA: halves the DMA count
                        wslab = wp.tile([128, 2, CT * 128], bf16, tag="wslab",
                                        bufs=3, name="wslab")
                        nc.sync.dma_start(
                            wslab[:], wsrc[:, 2 * mt2:2 * mt2 + 2, :])
                        for j in range(2):
                            mt = 2 * mt2 + j
                            pp = psp.tile([128, TCH], f32, tag="big", bufs=2,
                                          name="pp")
                            for kt in range(CT):
                                nc.tensor.matmul(
                                    pp[:],
                                    wslab[:, j, kt * 128:(kt + 1) * 128],
                                    x_sb[:, kt * TCH:(kt + 1) * TCH],
                                    start=(kt == 0), stop=(kt == CT - 1))
                            # v-evacs on ACT to relieve the DVE queue
                            ev_dst = dst[:, :, mt * GRP:(mt + 1) * GRP]
                            ev_src = pp[:].rearrange("p (g tj) -> p g tj",
                                                     tj=GRP)
                            if wname == "v":
                                nc.scalar.copy(ev_dst, ev_src)
                            else:
                                nc.vector.tensor_copy(ev_dst, ev_src)
                return pk

            def attn_chunk(t, pk):
                """Cross-head attention + output projection for one chunk."""
                qpk, kpk, vpk = pk["q"], pk["k"], pk["v"]
                at = aw.tile([128, CT, TCH], bf16, tag="at", bufs=1, name="atsb")
                st = {}

                def stage1(m):
                    ps_s = psp.tile([128, 512], f32, tag="s", bufs=2, name="ps_s")
                    for i in range(4):
                        g = 4 * m + i
                        nc.tensor.matmul(ps_s[:, i * 128:(i + 1) * 128],
                                         kpk[:, g, :], qpk[:, g, :],
                                         start=True, stop=True)
                    wt0 = aw.tile([128, 512], bf16, tag="wt0", bufs=3, name="wt0")
                    nc.scalar.activation(wt0[:], ps_s[:], EXP)
                    st[("wt0", m)] = wt0

                def stage1b(m):
                    # mask on Pool (SBUF-only engine) to offload DVE/ACT
                    wt0 = st.pop(("wt0", m))
                    wt = aw.tile([128, 512], bf16, tag="wt", bufs=3, name="wt")
                    nc.gpsimd.tensor_mul(wt[:], wt0[:], mask_sb[:])
                    st[("wt", m)] = wt

                def stage2(m):
                    wt = st[("wt", m)]
                    zt = psp.tile([128, TCH], f32, tag="big", bufs=2, name="zt")
                    for i in range(4):
                        nc.tensor.matmul(zt[:, i:i + 1],
                                         wt[:, i * 128:(i + 1) * 128], ones_sb[:],
                                         start=True, stop=True)
                    rz = aw.tile([128, 4], f32, tag="rz", bufs=3, name="rz")
                    nc.vector.reciprocal(rz[:], zt[:, :4])
                    st[("rz", m)] = rz
                    ps_v = psp.tile([128, 512], bf16, tag="v", bufs=1, name="ps_v")
                    for i in range(4):
                        g = 4 * m + i
                        nc.tensor.transpose(ps_v[:, i * 128:(i + 1) * 128],
                                            vpk[:, g, :], ident_sb[:])
                    vp = aw.tile([128, 512], bf16, tag="vp", bufs=3, name="vp")
                    nc.vector.tensor_copy(vp[:], ps_v[:])
                    st[("vp", m)] = vp

                def stage3(m):
                    wt = st.pop(("wt", m))
                    vp = st.pop(("vp", m))
                    rz = st.pop(("rz", m))
                    ps_at = psp.tile([128, 512], f32, tag="pat", bufs=2,
                                     name="ps_at")
                    for i in range(4):
                        nc.tensor.matmul(ps_at[:, i * 128:(i + 1) * 128],
                                         wt[:, i * 128:(i + 1) * 128],
                                         vp[:, i * 128:(i + 1) * 128],
                                         start=True, stop=True)
                    an = aw.tile([128, 512], bf16, tag="an", bufs=3, name="an")
                    nc.vector.tensor_mul(
                        an[:].rearrange("p (g c) -> p g c", g=4),
                        ps_at[:].rearrange("p (g c) -> p g c", g=4),
                        rz[:].broadcast_to((128, 4, 128)))
                    st[("an", m)] = an

                def stage4(m):
                    an = st.pop(("an", m))
                    ps_aT = psp.tile([128, 512], bf16, tag="aT", bufs=1,
                                     name="ps_aT")
                    for i in range(4):
                        nc.tensor.transpose(ps_aT[:, i * 128:(i + 1) * 128],
                                            an[:, i * 128:(i + 1) * 128],
                                            ident_sb[:])
                    # evac to at[d, h, tok] on ACT
                    nc.scalar.copy(
                        at[:, :, m * MAC:(m + 1) * MAC].rearrange(
                            "p h (g ti) -> p g h ti", ti=GRP),
                        ps_aT[:].rearrange(
                            "p (g h ti) -> p g h ti", g=4, h=H))

                for m in range(NMAC + 4):
                    if m < NMAC:
                        stage1(m)
                    if 1 <= m <= NMAC:
                        stage1b(m - 1)
                    if 2 <= m <= NMAC + 1:
                        stage2(m - 2)
                    if 3 <= m <= NMAC + 2:
                        stage3(m - 3)
                    if 4 <= m <= NMAC + 3:
                        stage4(m - 4)

                # output projection for this chunk
                for rt in range(CT):
                    woslab = wp.tile([128, CT * 128], bf16, tag="woslab",
                                     bufs=3, name="woslab")
                    nc.sync.dma_start(woslab[:], wo4[:, rt, :])
                    po = psp.tile([128, TCH], f32, tag="big", bufs=2, name="po")
                    for kt in range(CT):
                        nc.tensor.matmul(
                            po[:],
                            woslab[:, kt * 128:(kt + 1) * 128],
                            at[:, kt, :],
                            start=(kt == 0), stop=(kt == CT - 1))
                    oev = aw.tile([128, TCH], bf16, tag="oev", bufs=3, name="oev")
                    nc.vector.tensor_copy(oev[:], po[:])
                    nc.gpsimd.dma_start(
                        otb[:, rt, t * TCH:(t + 1) * TCH], oev[:])

            pks = {0: proj_chunk(0), 1: proj_chunk(1)}
            attn_chunk(0, pks.pop(0))
            pks[2] = proj_chunk(2)
            attn_chunk(1, pks.pop(1))
            pks[3] = proj_chunk(3)
            attn_chunk(2, pks.pop(2))
            attn_chunk(3, pks.pop(3))

    nc.compile()
    return nc


def _host_prep(x, wq, wk, wv, wo):
    """Build per-core input maps (layout transforms + bf16 casts only)."""
    import ml_dtypes
    bf16 = ml_dtypes.bfloat16

    x2 = np.ascontiguousarray(x.reshape(-1, HIDDEN))          # (16384, 2048)
    wqs = (wq / np.sqrt(np.float32(HD))).astype(np.float32)

    def wt4(w):   # [128, 16, 2048]: wt4[p, mt, kt*128+j] = w[mt*128+j, kt*128+p]
        return np.ascontiguousarray(
            w.reshape(H, 128, CT, 128).transpose(3, 0, 2, 1)
        ).reshape(128, H, CT * 128).astype(bf16)

    wq4, wk4, wv4, wo4 = wt4(wqs), wt4(wk), wt4(wv), wt4(wo)
    p = np.arange(128)[:, None]
    n = np.arange(128)[None, :]
    mask = np.where((p % GRP) == (n % GRP), 1.0, 0.0).astype(bf16)
    mask = np.tile(mask, (1, 4))
    ident = np.eye(128, dtype=np.float32).astype(bf16)

    in_maps = []
    for c in range(N_CORES):
        xs = x2[c * T_CORE:(c + 1) * T_CORE]                  # (2048, 2048)
        xtc = np.ascontiguousarray(
            xs.reshape(NTCH, TCH, CT, 128).transpose(3, 0, 2, 1)
        ).reshape(128, NTCH, CT * TCH).astype(bf16)
        in_maps.append({"xt": xtc, "wq4": wq4, "wk4": wk4, "wv4": wv4,
                        "wo4": wo4, "maskd": mask, "identd": ident})
    return in_maps


def kernel(x, wq, wk, wv, wo, inv_freq):
    # inv_freq is unused: RoPE is an identical orthogonal transform on q and k
    # at equal positions, and this attention only contracts same-position q·k,
    # so it cancels exactly.
    from concourse.bass_utils import run_bass_kernel_spmd

    x = np.asarray(x, dtype=np.float32)
    wq = np.asarray(wq, dtype=np.float32)
    wk = np.asarray(wk, dtype=np.float32)
    wv = np.asarray(wv, dtype=np.float32)
    wo = np.asarray(wo, dtype=np.float32)

    if "nc" not in _CACHED:
        _CACHED["nc"] = _build()
    nc = _CACHED["nc"]

    in_maps = _host_prep(x, wq, wk, wv, wo)
    res = run_bass_kernel_spmd(nc, in_maps, core_ids=list(range(N_CORES)))

    out = np.empty((N_CORES * T_CORE, HIDDEN), dtype=np.float32)
    for c in range(N_CORES):
        ot = np.asarray(res.results[c]["otb"]).astype(np.float32)  # (128,16,2048)
        out[c * T_CORE:(c + 1) * T_CORE] = (
            ot.transpose(2, 1, 0).reshape(T_CORE, HIDDEN))
    return out.reshape(x.shape[0], x.shape[1], HIDDEN)


# revision 19
# speedup vs baseline: 1.1737x; 1.0466x over previous
"""Trainium2 Bass kernel for nn_LlamaAttention_6588479832091.

Math notes:
  - The reference attention contracts q and k at the SAME sequence position
    (scores = einsum('bshd,bstd->bsht', q, k)), and RoPE applies the same
    orthogonal transform to q and k at equal positions, so RoPE cancels
    exactly: (P R q)·(P R k) = q·k.  v and the output path never see RoPE.
    The kernel therefore computes: q/k/v projections, per-token 16x16
    cross-head softmax attention, and the output projection.
  - Sharding: data-parallel over the 16384 tokens -> 2048 tokens per core,
    weights replicated.  No collectives.
  - All matmuls run in bf16 (1 cycle/row on the PE; fp32 would be 4) with
    fp32 PSUM accumulation.  End-to-end rel err ~5e-3, tolerance is 2e-2.
  - Fully fused per-512-token-chunk pipeline: the q/k/v projection psums are
    evacuated DIRECTLY into the attention's group-packed SBUF layout (no
    DRAM roundtrip, no staging loads).  Weight slabs are re-streamed per
    chunk instead (DMA is far below the PE roofline).  Emission order
    proj(0), proj(1), A(0), proj(2), A(1), proj(3), A(2), A(3) keeps the
    PE busy across chunk boundaries.
  - Attention softmax work is spread over DVE/ACT/Pool so no single engine
    exceeds the PE's per-macro cadence: exp on ACT, mask-mul + recip +
    normalize on DVE, v-transpose evac on Pool, attn-transpose evac split
    ACT/Pool.  Mask is multiplicative (0/1) applied to exp(scores); scores
    are O(few) so exp never overflows.

Layouts (host-prepared, all partition-first, bf16):
  xt   [128, 4, 8192]   xt[p, t, kt*512+i] = x_shard[t*512+i, kt*128+p]
  wq4  [128, 16, 2048]  wq4[p, mt, kt*128+j] = wq[mt*128+j, kt*128+p]/sqrt(128)
  wk4, wv4: same layout as wq4 (wk, wv unscaled)
  wo4  [128, 16, 2048]  wo4[p, rt, kt*128+j] = wo[rt*128+j, kt*128+p]
  maskd [128, 512]      1 where p%8 == n%8 else 0 (tiled x4 groups)
  identd [128, 128]     identity
  otb  [128, 16, 2048]  otb[p, rt, t] = out_shard[t, rt*128+p]   (output)
"""
import sys

for _p in ("/opt/trn_rl_repo", "/root/.axon_site/_ro/trn_rl_repo"):
    if _p not in sys.path:
        sys.path.insert(0, _p)

import numpy as np

T_CORE = 2048      # tokens per core
N_CORES = 8
H = 16             # heads
HD = 128           # head dim
HIDDEN = 2048
CT = HIDDEN // 128  # 16 contraction tiles
TCH = 512          # tokens per fused chunk
NTCH = T_CORE // TCH  # 4 chunks
GRP = 8            # tokens per attention group
NG = TCH // GRP    # 64 groups per chunk
MAC = 32           # tokens per macro (4 groups)
NMAC = TCH // MAC  # 16 macros per chunk

_CACHED = {}


def _build():
    import concourse.mybir as mybir
    import concourse.tile as tile
    import concourse.bacc as bacc

    f32 = mybir.dt.float32
    bf16 = mybir.dt.bfloat16
    EXP = mybir.ActivationFunctionType.Exp

    nc = bacc.Bacc("TRN2", target_bir_lowering=False, debug=False)

    xt = nc.declare_dram_parameter("xt", [128, NTCH, CT * TCH], bf16, isOutput=False)
    wq4 = nc.declare_dram_parameter("wq4", [128, H, CT * 128], bf16, isOutput=False)
    wk4 = nc.declare_dram_parameter("wk4", [128, H, CT * 128], bf16, isOutput=False)
    wv4 = nc.declare_dram_parameter("wv4", [128, H, CT * 128], bf16, isOutput=False)
    wo4 = nc.declare_dram_parameter("wo4", [128, CT, CT * 128], bf16, isOutput=False)
    maskd = nc.declare_dram_parameter("maskd", [128, 512], bf16, isOutput=False)
    identd = nc.declare_dram_parameter("identd", [128, 128], bf16, isOutput=False)
    otb = nc.declare_dram_parameter("otb", [128, CT, T_CORE], bf16, isOutput=True)

    with tile.TileContext(nc) as tc:
        with tc.tile_pool(name="io", bufs=1) as io, \
             tc.tile_pool(name="wp", bufs=1) as wp, \
             tc.tile_pool(name="xp", bufs=1) as xp, \
             tc.tile_pool(name="qk", bufs=1) as qkp, \
             tc.tile_pool(name="aw", bufs=1) as aw, \
             tc.tile_pool(name="ps", bufs=1, space="PSUM") as psp:

            mask_sb = io.tile([128, 512], bf16, name="masksb")
            ident_sb = io.tile([128, 128], bf16, name="identsb")
            ones_sb = io.tile([128, 1], bf16, name="onessb")
            nc.scalar.dma_start(mask_sb[:], maskd[:])
            nc.scalar.dma_start(ident_sb[:], identd[:])
            nc.gpsimd.memset(ones_sb[:], 1.0)

            def make_proj(t):
                """q/k/v projections for 512 tokens, evacuated straight into
                the attention's packed layout [128=d, group, (h, tj)].
                Returns (pk dict, generator yielding after each psum-group)."""
                x_sb = xp.tile([128, CT * TCH], bf16, tag="x", bufs=2, name="xsb")
                # x on the ACT DMA queue (parallel with slab loads on sync);
                # split so the first psum's kt-chain can start early
                for piece in range(4):
                    sl = slice(piece * 4 * TCH, (piece + 1) * 4 * TCH)
                    nc.scalar.dma_start(x_sb[:, sl], xt[:, t, sl])
                pk = {}
                for wname in ("q", "k", "v"):
                    pk[wname] = qkp.tile([128, NG, 128], bf16, tag=f"{wname}pk",
                                         bufs=2, name=f"{wname}pk")

                def gen():
                    for wname, wsrc in (("q", wq4), ("k", wk4), ("v", wv4)):
                        dst = pk[wname]
                        for mt2 in range(H // 2):
                            # two head-slabs per DMA: halves the DMA count
                            wslab = wp.tile([128, 2, CT * 128], bf16,
                                            tag="wslab", bufs=2, name="wslab")
                            nc.sync.dma_start(
                                wslab[:], wsrc[:, 2 * mt2:2 * mt2 + 2, :])
                            for j in range(2):
                                mt = 2 * mt2 + j
                                pp = psp.tile([128, TCH], f32, tag="big",
                                              bufs=2, name="pp")
                                for kt in range(CT):
                                    nc.tensor.matmul(
                                        pp[:],
                                        wslab[:, j, kt * 128:(kt + 1) * 128],
                                        x_sb[:, kt * TCH:(kt + 1) * TCH],
                                        start=(kt == 0), stop=(kt == CT - 1))
                                # v-evacs on ACT to relieve the DVE queue
                                ev_dst = dst[:, :, mt * GRP:(mt + 1) * GRP]
                                ev_src = pp[:].rearrange(
                                    "p (g tj) -> p g tj", tj=GRP)
                                if wname == "v":
                                    nc.scalar.copy(ev_dst, ev_src)
                                else:
                                    nc.vector.tensor_copy(ev_dst, ev_src)
                                yield
                return pk, gen()

            def make_attn(t, pk):
                """Cross-head attention macros for one chunk; emitted
                interleaved into PE-heavy windows so the softmax's DVE/ACT/
                Pool ops never outrun the PE. Returns (at tile, generator)."""
                qpk, kpk, vpk = pk["q"], pk["k"], pk["v"]
                at = aw.tile([128, CT, TCH], bf16, tag="at", bufs=2, name="atsb")
                st = {}

                def stage1(m):
                    ps_s = psp.tile([128, 512], f32, tag="s", bufs=2, name="ps_s")
                    for i in range(4):
                        g = 4 * m + i
                        nc.tensor.matmul(ps_s[:, i * 128:(i + 1) * 128],
                                         kpk[:, g, :], qpk[:, g, :],
                                         start=True, stop=True)
                    wt0 = aw.tile([128, 512], bf16, tag="wt0", bufs=3, name="wt0")
                    nc.scalar.activation(wt0[:], ps_s[:], EXP)
                    st[("wt0", m)] = wt0

                def stage1b(m):
                    # mask on Pool (SBUF-only engine) to offload DVE/ACT
                    wt0 = st.pop(("wt0", m))
                    wt = aw.tile([128, 512], bf16, tag="wt", bufs=3, name="wt")
                    nc.gpsimd.tensor_mul(wt[:], wt0[:], mask_sb[:])
                    st[("wt", m)] = wt

                def stage2(m):
                    wt = st[("wt", m)]
                    zt = psp.tile([128, TCH], f32, tag="big", bufs=2, name="zt")
                    for i in range(4):
                        nc.tensor.matmul(zt[:, i:i + 1],
                                         wt[:, i * 128:(i + 1) * 128], ones_sb[:],
                                         start=True, stop=True)
                    rz = aw.tile([128, 4], f32, tag="rz", bufs=3, name="rz")
                    nc.vector.reciprocal(rz[:], zt[:, :4])
                    st[("rz", m)] = rz
                    ps_v = psp.tile([128, 512], bf16, tag="v", bufs=1, name="ps_v")
                    for i in range(4):
                        g = 4 * m + i
                        nc.tensor.transpose(ps_v[:, i * 128:(i + 1) * 128],
                                            vpk[:, g, :], ident_sb[:])
                    vp = aw.tile([128, 512], bf16, tag="vp", bufs=3, name="vp")
                    nc.vector.tensor_copy(vp[:], ps_v[:])
                    st[("vp", m)] = vp

                def stage3(m):
                    wt = st.pop(("wt", m))
                    vp = st.pop(("vp", m))
                    rz = st.pop(("rz", m))
                    ps_at = psp.tile([128, 512], f32, tag="pat", bufs=2,
                                     name="ps_at")
                    for i in range(4):
                        nc.tensor.matmul(ps_at[:, i * 128:(i + 1) * 128],
                                         wt[:, i * 128:(i + 1) * 128],
                                         vp[:, i * 128:(i + 1) * 128],
                                         start=True, stop=True)
                    an = aw.tile([128, 512], bf16, tag="an", bufs=3, name="an")
                    nc.vector.tensor_mul(
                        an[:].rearrange("p (g c) -> p g c", g=4),
                        ps_at[:].rearrange("p (g c) -> p g c", g=4),
                        rz[:].broadcast_to((128, 4, 128)))
                    st[("an", m)] = an

                def stage4(m):
                    an = st.pop(("an", m))
                    ps_aT = psp.tile([128, 512], bf16, tag="aT", bufs=1,
                                     name="ps_aT")
                    for i in range(4):
                        nc.tensor.transpose(ps_aT[:, i * 128:(i + 1) * 128],
                                            an[:, i * 128:(i + 1) * 128],
                                            ident_sb[:])
                    # evac to at[d, h, tok] on ACT
                    nc.scalar.copy(
                        at[:, :, m * MAC:(m + 1) * MAC].rearrange(
                            "p h (g ti) -> p g h ti", ti=GRP),
                        ps_aT[:].rearrange(
                            "p (g h ti) -> p g h ti", g=4, h=H))

                def gen():
                    for m in range(NMAC + 4):
                        if m < NMAC:
                            stage1(m)
                        if 1 <= m <= NMAC:
                            stage1b(m - 1)
                        if 2 <= m <= NMAC + 1:
                            stage2(m - 2)
                        if 3 <= m <= NMAC + 2:
                            stage3(m - 3)
                        if 4 <= m <= NMAC + 3:
                            stage4(m - 4)
                        yield
                return at, gen()

            def make_oproj(t, at):
                """Output projection generator, one yield per rt group."""
                def gen():
                    for rt2 in range(CT // 2):
                        woslab = wp.tile([128, 2, CT * 128], bf16,
                                         tag="woslab", bufs=2, name="woslab")
                        nc.sync.dma_start(
                            woslab[:], wo4[:, 2 * rt2:2 * rt2 + 2, :])
                        for j in range(2):
                            rt = 2 * rt2 + j
                            po = psp.tile([128, TCH], f32, tag="big", bufs=2,
                                          name="po")
                            for kt in range(CT):
                                nc.tensor.matmul(
                                    po[:],
                                    woslab[:, j, kt * 128:(kt + 1) * 128],
                                    at[:, kt, :],
                                    start=(kt == 0), stop=(kt == CT - 1))
                            oev = aw.tile([128, TCH], bf16, tag="oev", bufs=2,
                                          name="oev")
                            nc.vector.tensor_copy(oev[:], po[:])
                            nc.gpsimd.dma_start(
                                otb[:, rt, t * TCH:(t + 1) * TCH], oev[:])
                            yield
                return gen()

            def interleave(gen_a, na, gen_b, nb):
                """Emit gen_a's units with gen_b's rate-matched in between."""
                done_b = 0
                for i in range(na):
                    next(gen_a)
                    want = (i + 1) * nb // na
                    while done_b < want:
                        next(gen_b)
                        done_b += 1
                for _ in gen_a:
                    pass
                for _ in gen_b:
                    pass

            def drain(g):
                for _ in g:
                    pass

            # schedule: P0; P1(+)A0; O0(+)A1; P2; O1(+)A2; P3; O2(+)A3; O3
            pk0, pg0 = make_proj(0)
            drain(pg0)
            pk1, pg1 = make_proj(1)
            at0, ag0 = make_attn(0, pk0)
            interleave(pg1, 48, ag0, NMAC + 4)
            og0 = make_oproj(0, at0)
            at1, ag1 = make_attn(1, pk1)
            interleave(og0, CT, ag1, NMAC + 4)
            pk2, pg2 = make_proj(2)
            drain(pg2)
            og1 = make_oproj(1, at1)
            at2, ag2 = make_attn(2, pk2)
            interleave(og1, CT, ag2, NMAC + 4)
            pk3, pg3 = make_proj(3)
            drain(pg3)
            og2 = make_oproj(2, at2)
            at3, ag3 = make_attn(3, pk3)
            interleave(og2, CT, ag3, NMAC + 4)
            og3 = make_oproj(3, at3)
            drain(og3)

    nc.compile()
    return nc


def _host_prep(x, wq, wk, wv, wo):
    """Build per-core input maps (layout transforms + bf16 casts only)."""
    import ml_dtypes
    bf16 = ml_dtypes.bfloat16

    x2 = np.ascontiguousarray(x.reshape(-1, HIDDEN))          # (16384, 2048)
    wqs = (wq / np.sqrt(np.float32(HD))).astype(np.float32)

    def wt4(w):   # [128, 16, 2048]: wt4[p, mt, kt*128+j] = w[mt*128+j, kt*128+p]
        return np.ascontiguousarray(
            w.reshape(H, 128, CT, 128).transpose(3, 0, 2, 1)
        ).reshape(128, H, CT * 128).astype(bf16)

    wq4, wk4, wv4, wo4 = wt4(wqs), wt4(wk), wt4(wv), wt4(wo)
    p = np.arange(128)[:, None]
    n = np.arange(128)[None, :]
    mask = np.where((p % GRP) == (n % GRP), 1.0, 0.0).astype(bf16)
    mask = np.tile(mask, (1, 4))
    ident = np.eye(128, dtype=np.float32).astype(bf16)

    in_maps = []
    for c in range(N_CORES):
        xs = x2[c * T_CORE:(c + 1) * T_CORE]                  # (2048, 2048)
        xtc = np.ascontiguousarray(
            xs.reshape(NTCH, TCH, CT, 128).transpose(3, 0, 2, 1)
        ).reshape(128, NTCH, CT * TCH).astype(bf16)
        in_maps.append({"xt": xtc, "wq4": wq4, "wk4": wk4, "wv4": wv4,
                        "wo4": wo4, "maskd": mask, "identd": ident})
    return in_maps


def kernel(x, wq, wk, wv, wo, inv_freq):
    # inv_freq is unused: RoPE is an identical orthogonal transform on q and k
    # at equal positions, and this attention only contracts same-position q·k,
    # so it cancels exactly.
    from concourse.bass_utils import run_bass_kernel_spmd

    x = np.asarray(x, dtype=np.float32)
    wq = np.asarray(wq, dtype=np.float32)
    wk = np.asarray(wk, dtype=np.float32)
    wv = np.asarray(wv, dtype=np.float32)
    wo = np.asarray(wo, dtype=np.float32)

    if "nc" not in _CACHED:
        _CACHED["nc"] = _build()
    nc = _CACHED["nc"]

    in_maps = _host_prep(x, wq, wk, wv, wo)
    res = run_bass_kernel_spmd(nc, in_maps, core_ids=list(range(N_CORES)))

    out = np.empty((N_CORES * T_CORE, HIDDEN), dtype=np.float32)
    for c in range(N_CORES):
        ot = np.asarray(res.results[c]["otb"]).astype(np.float32)  # (128,16,2048)
        out[c * T_CORE:(c + 1) * T_CORE] = (
            ot.transpose(2, 1, 0).reshape(T_CORE, HIDDEN))
    return out.reshape(x.shape[0], x.shape[1], HIDDEN)


# revision 24
# speedup vs baseline: 1.1744x; 1.0006x over previous
"""Trainium2 Bass kernel for nn_LlamaAttention_6588479832091.

Math notes:
  - The reference attention contracts q and k at the SAME sequence position
    (scores = einsum('bshd,bstd->bsht', q, k)), and RoPE applies the same
    orthogonal transform to q and k at equal positions, so RoPE cancels
    exactly: (P R q)·(P R k) = q·k.  v and the output path never see RoPE.
    The kernel therefore computes: q/k/v projections, per-token 16x16
    cross-head softmax attention, and the output projection.
  - Sharding: data-parallel over the 16384 tokens -> 2048 tokens per core,
    weights replicated.  No collectives.
  - All matmuls run in bf16 (1 cycle/row on the PE; fp32 would be 4) with
    fp32 PSUM accumulation.  End-to-end rel err ~5e-3, tolerance is 2e-2.
  - Fully fused per-512-token-chunk pipeline: the q/k/v projection psums are
    evacuated DIRECTLY into the attention's group-packed SBUF layout (no
    DRAM roundtrip, no staging loads).  Weight slabs are re-streamed per
    chunk instead (DMA is far below the PE roofline).  Emission order
    proj(0), proj(1), A(0), proj(2), A(1), proj(3), A(2), A(3) keeps the
    PE busy across chunk boundaries.
  - Attention softmax work is spread over DVE/ACT/Pool so no single engine
    exceeds the PE's per-macro cadence: exp on ACT, mask-mul + recip +
    normalize on DVE, v-transpose evac on Pool, attn-transpose evac split
    ACT/Pool.  Mask is multiplicative (0/1) applied to exp(scores); scores
    are O(few) so exp never overflows.

Layouts (host-prepared, all partition-first, bf16):
  xt   [128, 4, 8192]   xt[p, t, kt*512+i] = x_shard[t*512+i, kt*128+p]
  wq4  [128, 16, 2048]  wq4[p, mt, kt*128+j] = wq[mt*128+j, kt*128+p]/sqrt(128)
  wk4, wv4: same layout as wq4 (wk, wv unscaled)
  wo4  [128, 16, 2048]  wo4[p, rt, kt*128+j] = wo[rt*128+j, kt*128+p]
  maskd [128, 512]      1 where p%8 == n%8 else 0 (tiled x4 groups)
  identd [128, 128]     identity
  otb  [128, 16, 2048]  otb[p, rt, t] = out_shard[t, rt*128+p]   (output)
"""
import sys

for _p in ("/opt/trn_rl_repo", "/root/.axon_site/_ro/trn_rl_repo"):
    if _p not in sys.path:
        sys.path.insert(0, _p)

import numpy as np

T_CORE = 2048      # tokens per core
N_CORES = 8
H = 16             # heads
HD = 128           # head dim
HIDDEN = 2048
CT = HIDDEN // 128  # 16 contraction tiles
TCH = 512          # tokens per fused chunk
NTCH = T_CORE // TCH  # 4 chunks
GRP = 8            # tokens per attention group
NG = TCH // GRP    # 64 groups per chunk
MAC = 32           # tokens per macro (4 groups)
NMAC = TCH // MAC  # 16 macros per chunk

_CACHED = {}


def _build():
    import concourse.mybir as mybir
    import concourse.tile as tile
    import concourse.bacc as bacc

    f32 = mybir.dt.float32
    bf16 = mybir.dt.bfloat16
    EXP = mybir.ActivationFunctionType.Exp

    nc = bacc.Bacc("TRN2", target_bir_lowering=False, debug=False)

    xt = nc.declare_dram_parameter("xt", [128, NTCH, CT * TCH], bf16, isOutput=False)
    wq4 = nc.declare_dram_parameter("wq4", [128, H, CT * 128], bf16, isOutput=False)
    wk4 = nc.declare_dram_parameter("wk4", [128, H, CT * 128], bf16, isOutput=False)
    wv4 = nc.declare_dram_parameter("wv4", [128, H, CT * 128], bf16, isOutput=False)
    wo4 = nc.declare_dram_parameter("wo4", [128, CT, CT * 128], bf16, isOutput=False)
    maskd = nc.declare_dram_parameter("maskd", [128, 512], bf16, isOutput=False)
    identd = nc.declare_dram_parameter("identd", [128, 128], bf16, isOutput=False)
    otb = nc.declare_dram_parameter("otb", [128, CT, T_CORE], bf16, isOutput=True)

    with tile.TileContext(nc) as tc:
        with tc.tile_pool(name="io", bufs=1) as io, \
             tc.tile_pool(name="wp", bufs=1) as wp, \
             tc.tile_pool(name="xp", bufs=1) as xp, \
             tc.tile_pool(name="qk", bufs=1) as qkp, \
             tc.tile_pool(name="aw", bufs=1) as aw, \
             tc.tile_pool(name="ps", bufs=1, space="PSUM") as psp:

            mask_sb = io.tile([128, 512], bf16, name="masksb")
            ident_sb = io.tile([128, 128], bf16, name="identsb")
            ones_sb = io.tile([128, 1], bf16, name="onessb")
            nc.gpsimd.memset(ones_sb[:], 1.0)

            def make_proj(t):
                """q/k/v projections for 512 tokens, evacuated straight into
                the attention's packed layout [128=d, group, (h, tj)].
                Returns (pk dict, generator yielding after each psum-group)."""
                x_sb = xp.tile([128, CT * TCH], bf16, tag="x", bufs=2, name="xsb")
                # x on the ACT DMA queue (parallel with slab loads on sync);
                # split so the first psum's kt-chain can start early; chunk 0
                # gets finer early pieces since nothing hides its latency
                bounds = (0, 2, 4, 8, 16) if t == 0 else (0, 4, 8, 12, 16)
                for piece in range(4):
                    sl = slice(bounds[piece] * TCH, bounds[piece + 1] * TCH)
                    nc.scalar.dma_start(x_sb[:, sl], xt[:, t, sl])
                pk = {}
                for wname in ("q", "k", "v"):
                    pk[wname] = qkp.tile([128, NG, 128], bf16, tag=f"{wname}pk",
                                         bufs=2, name=f"{wname}pk")

                def gen():
                    for wname, wsrc in (("q", wq4), ("k", wk4), ("v", wv4)):
                        dst = pk[wname]
                        for mt2 in range(H // 2):
                            # two head-slabs per DMA: halves the DMA count
                            wslab = wp.tile([128, 2, CT * 128], bf16,
                                            tag="wslab", bufs=2, name="wslab")
                            if t == 0 and wname == "q" and mt2 == 0:
                                # two singles so the very first matmul group
                                # waits on half the transfer
                                nc.sync.dma_start(wslab[:, 0, :], wsrc[:, 0, :])
                                nc.sync.dma_start(wslab[:, 1, :], wsrc[:, 1, :])
                            else:
                                nc.sync.dma_start(
                                    wslab[:], wsrc[:, 2 * mt2:2 * mt2 + 2, :])
                            for j in range(2):
                                mt = 2 * mt2 + j
                                pp = psp.tile([128, TCH], f32, tag="big",
                                              bufs=2, name="pp")
                                for kt in range(CT):
                                    nc.tensor.matmul(
                                        pp[:],
                                        wslab[:, j, kt * 128:(kt + 1) * 128],
                                        x_sb[:, kt * TCH:(kt + 1) * TCH],
                                        start=(kt == 0), stop=(kt == CT - 1))
                                # v-evacs on ACT to relieve the DVE queue
                                ev_dst = dst[:, :, mt * GRP:(mt + 1) * GRP]
                                ev_src = pp[:].rearrange(
                                    "p (g tj) -> p g tj", tj=GRP)
                                if wname == "v":
                                    nc.scalar.copy(ev_dst, ev_src)
                                else:
                                    nc.vector.tensor_copy(ev_dst, ev_src)
                                yield
                return pk, gen()

            def make_attn(t, pk):
                """Cross-head attention macros for one chunk; emitted
                interleaved into PE-heavy windows so the softmax's DVE/ACT/
                Pool ops never outrun the PE. Returns (at tile, generator)."""
                qpk, kpk, vpk = pk["q"], pk["k"], pk["v"]
                at = aw.tile([128, CT, TCH], bf16, tag="at", bufs=2, name="atsb")
                st = {}

                def stage1(m):
                    ps_s = psp.tile([128, 512], f32, tag="s", bufs=2, name="ps_s")
                    for i in range(4):
                        g = 4 * m + i
                        nc.tensor.matmul(ps_s[:, i * 128:(i + 1) * 128],
                                         kpk[:, g, :], qpk[:, g, :],
                                         start=True, stop=True)
                    wt0 = aw.tile([128, 512], bf16, tag="wt0", bufs=3, name="wt0")
                    nc.scalar.activation(wt0[:], ps_s[:], EXP)
                    st[("wt0", m)] = wt0

                def stage1b(m):
                    # mask on Pool (SBUF-only engine) to offload DVE/ACT
                    wt0 = st.pop(("wt0", m))
                    wt = aw.tile([128, 512], bf16, tag="wt", bufs=3, name="wt")
                    nc.gpsimd.tensor_mul(wt[:], wt0[:], mask_sb[:])
                    st[("wt", m)] = wt

                def stage2(m):
                    wt = st[("wt", m)]
                    zt = psp.tile([128, TCH], f32, tag="big", bufs=2, name="zt")
                    for i in range(4):
                        nc.tensor.matmul(zt[:, i:i + 1],
                                         wt[:, i * 128:(i + 1) * 128], ones_sb[:],
                                         start=True, stop=True)
                    rz = aw.tile([128, 4], f32, tag="rz", bufs=3, name="rz")
                    nc.vector.reciprocal(rz[:], zt[:, :4])
                    st[("rz", m)] = rz
                    ps_v = psp.tile([128, 512], bf16, tag="v", bufs=1, name="ps_v")
                    for i in range(4):
                        g = 4 * m + i
                        nc.tensor.transpose(ps_v[:, i * 128:(i + 1) * 128],
                                            vpk[:, g, :], ident_sb[:])
                    vp = aw.tile([128, 512], bf16, tag="vp", bufs=3, name="vp")
                    nc.vector.tensor_copy(vp[:], ps_v[:])
                    st[("vp", m)] = vp

                def stage3(m):
                    wt = st.pop(("wt", m))
                    vp = st.pop(("vp", m))
                    rz = st.pop(("rz", m))
                    ps_at = psp.tile([128, 512], f32, tag="pat", bufs=2,
                                     name="ps_at")
                    for i in range(4):
                        nc.tensor.matmul(ps_at[:, i * 128:(i + 1) * 128],
                                         wt[:, i * 128:(i + 1) * 128],
                                         vp[:, i * 128:(i + 1) * 128],
                                         start=True, stop=True)
                    an = aw.tile([128, 512], bf16, tag="an", bufs=3, name="an")
                    nc.vector.tensor_mul(
                        an[:].rearrange("p (g c) -> p g c", g=4),
                        ps_at[:].rearrange("p (g c) -> p g c", g=4),
                        rz[:].broadcast_to((128, 4, 128)))
                    st[("an", m)] = an

                def stage4(m):
                    an = st.pop(("an", m))
                    ps_aT = psp.tile([128, 512], bf16, tag="aT", bufs=1,
                                     name="ps_aT")
                    for i in range(4):
                        nc.tensor.transpose(ps_aT[:, i * 128:(i + 1) * 128],
                                            an[:, i * 128:(i + 1) * 128],
                                            ident_sb[:])
                    # evac to at[d, h, tok] on ACT
                    nc.scalar.copy(
                        at[:, :, m * MAC:(m + 1) * MAC].rearrange(
                            "p h (g ti) -> p g h ti", ti=GRP),
                        ps_aT[:].rearrange(
                            "p (g h ti) -> p g h ti", g=4, h=H))

                def gen():
                    for m in range(NMAC + 4):
                        if m < NMAC:
                            stage1(m)
                        if 1 <= m <= NMAC:
                            stage1b(m - 1)
                        if 2 <= m <= NMAC + 1:
                            stage2(m - 2)
                        if 3 <= m <= NMAC + 2:
                            stage3(m - 3)
                        if 4 <= m <= NMAC + 3:
                            stage4(m - 4)
                        yield
                return at, gen()

            def make_oproj(t, at):
                """Output projection generator, one yield per rt group."""
                def gen():
                    for rt2 in range(CT // 2):
                        woslab = wp.tile([128, 2, CT * 128], bf16,
                                         tag="woslab", bufs=2, name="woslab")
                        nc.sync.dma_start(
                            woslab[:], wo4[:, 2 * rt2:2 * rt2 + 2, :])
                        for j in range(2):
                            rt = 2 * rt2 + j
                            po = psp.tile([128, TCH], f32, tag="big", bufs=2,
                                          name="po")
                            for kt in range(CT):
                                nc.tensor.matmul(
                                    po[:],
                                    woslab[:, j, kt * 128:(kt + 1) * 128],
                                    at[:, kt, :],
                                    start=(kt == 0), stop=(kt == CT - 1))
                            oev = aw.tile([128, TCH], bf16, tag="oev", bufs=2,
                                          name="oev")
                            nc.vector.tensor_copy(oev[:], po[:])
                            # last chunk: store via HWDGE (sync) — lower
                            # latency than SWDGE desc-gen, shortens the tail
                            eng = nc.sync if t == NTCH - 1 else nc.gpsimd
                            eng.dma_start(
                                otb[:, rt, t * TCH:(t + 1) * TCH], oev[:])
                            yield
                return gen()

            def interleave(gen_a, na, gen_b, nb):
                """Emit gen_a's units with gen_b's rate-matched in between."""
                done_b = 0
                for i in range(na):
                    next(gen_a)
                    want = (i + 1) * nb // na
                    while done_b < want:
                        next(gen_b)
                        done_b += 1
                for _ in gen_a:
                    pass
                for _ in gen_b:
                    pass

            def drain(g):
                for _ in g:
                    pass

            # schedule: P0; P1(+)A0; O0(+)A1; P2; O1(+)A2; P3; O2(+)A3; O3
            pk0, pg0 = make_proj(0)
            # mask/ident after chunk0's x pieces on the ACT queue (only
            # needed once attention starts)
            nc.scalar.dma_start(mask_sb[:], maskd[:])
            nc.scalar.dma_start(ident_sb[:], identd[:])
            drain(pg0)
            pk1, pg1 = make_proj(1)
            at0, ag0 = make_attn(0, pk0)
            interleave(pg1, 48, ag0, NMAC + 4)
            og0 = make_oproj(0, at0)
            at1, ag1 = make_attn(1, pk1)
            interleave(og0, CT, ag1, NMAC + 4)
            pk2, pg2 = make_proj(2)
            drain(pg2)
            og1 = make_oproj(1, at1)
            at2, ag2 = make_attn(2, pk2)
            interleave(og1, CT, ag2, NMAC + 4)
            pk3, pg3 = make_proj(3)
            drain(pg3)
            og2 = make_oproj(2, at2)
            at3, ag3 = make_attn(3, pk3)
            interleave(og2, CT, ag3, NMAC + 4)
            og3 = make_oproj(3, at3)
            drain(og3)

    nc.compile()
    return nc


def _host_prep(x, wq, wk, wv, wo):
    """Build per-core input maps (layout transforms + bf16 casts only)."""
    import ml_dtypes
    bf16 = ml_dtypes.bfloat16

    x2 = np.ascontiguousarray(x.reshape(-1, HIDDEN))          # (16384, 2048)
    wqs = (wq / np.sqrt(np.float32(HD))).astype(np.float32)

    def wt4(w):   # [128, 16, 2048]: wt4[p, mt, kt*128+j] = w[mt*128+j, kt*128+p]
        return np.ascontiguousarray(
            w.reshape(H, 128, CT, 128).transpose(3, 0, 2, 1)
        ).reshape(128, H, CT * 128).astype(bf16)

    wq4, wk4, wv4, wo4 = wt4(wqs), wt4(wk), wt4(wv), wt4(wo)
    p = np.arange(128)[:, None]
    n = np.arange(128)[None, :]
    mask = np.where((p % GRP) == (n % GRP), 1.0, 0.0).astype(bf16)
    mask = np.tile(mask, (1, 4))
    ident = np.eye(128, dtype=np.float32).astype(bf16)

    in_maps = []
    for c in range(N_CORES):
        xs = x2[c * T_CORE:(c + 1) * T_CORE]                  # (2048, 2048)
        xtc = np.ascontiguousarray(
            xs.reshape(NTCH, TCH, CT, 128).transpose(3, 0, 2, 1)
        ).reshape(128, NTCH, CT * TCH).astype(bf16)
        in_maps.append({"xt": xtc, "wq4": wq4, "wk4": wk4, "wv4": wv4,
                        "wo4": wo4, "maskd": mask, "identd": ident})
    return in_maps


def kernel(x, wq, wk, wv, wo, inv_freq):
    # inv_freq is unused: RoPE is an identical orthogonal transform on q and k
    # at equal positions, and this attention only contracts same-position q·k,
    # so it cancels exactly.
    from concourse.bass_utils import run_bass_kernel_spmd

    x = np.asarray(x, dtype=np.float32)
    wq = np.asarray(wq, dtype=np.float32)
    wk = np.asarray(wk, dtype=np.float32)
    wv = np.asarray(wv, dtype=np.float32)
    wo = np.asarray(wo, dtype=np.float32)

    if "nc" not in _CACHED:
        _CACHED["nc"] = _build()
    nc = _CACHED["nc"]

    in_maps = _host_prep(x, wq, wk, wv, wo)
    res = run_bass_kernel_spmd(nc, in_maps, core_ids=list(range(N_CORES)))

    out = np.empty((N_CORES * T_CORE, HIDDEN), dtype=np.float32)
    for c in range(N_CORES):
        ot = np.asarray(res.results[c]["otb"]).astype(np.float32)  # (128,16,2048)
        out[c * T_CORE:(c + 1) * T_CORE] = (
            ot.transpose(2, 1, 0).reshape(T_CORE, HIDDEN))
    return out.reshape(x.shape[0], x.shape[1], HIDDEN)
